# revision 1
# baseline (speedup 1.0000x reference)
"""Trainium2 Bass kernel for nn_LNKillingRelu.

Math (per batch b, channel g, point n; L=8 lie-algebra coords):
    d[b,g,:,n]  = sum_f W[g,f] * x[b,f,:,n]          (64x64 linear over channels)
    kf[b,g,n]   = x[b,g,:,n]^T K d[b,g,:,n]          (8x8 Killing bilinear form)
    out         = x + relu(kf) * d                    (broadcast kf over L)

K is a fixed sparse matrix: permutation pairs (0<->6, 1<->7, 2<->3) plus a
[[2,1],[1,2]] block on coords (4,5).

Sharding: data-parallel over batch B=16 -> 2 batches per core (8 cores).
Per-core layout: partitions = (batch-pair, F=64) = 128; free = (L, n-chunk).
PE computes d with a block-diag(W^T, W^T) 128x128 stationary weight (full PE
utilization, both batches contracted in one matmul stream). ACT copies
PSUM->SBUF and applies relu; DVE computes the Killing products + L-reduce;
GPSIMD takes the r*d multiply and part of the final add.
"""

import numpy as np
from contextlib import ExitStack

import concourse.bass as bass
import concourse.bacc as bacc
import concourse.tile as tile
from concourse import mybir
from concourse.bass_utils import run_bass_kernel_spmd

B, F, L, N = 16, 64, 8, 4096
N_CORES = 8
B_PER_CORE = B // N_CORES  # 2
NT = 256                   # points per chunk
N_CHUNKS = N // NT

F32 = mybir.dt.float32
MULT = mybir.AluOpType.mult
ADD = mybir.AluOpType.add


def _bcast_l(ap: bass.AP, l: int) -> bass.AP:
    """[128, NT] AP -> [128, l, NT] with a zero-stride middle dim."""
    return bass.AP(tensor=ap.tensor, offset=ap.offset,
                   ap=[ap.ap[0], [0, l], ap.ap[1]])


def _build(nt: int = NT, lsplit: int = 2, psum_bufs: int = 2):
    """lsplit: l-slices [0:lsplit) of the r*d multiply + final add run on DVE,
    the rest on GPSIMD (two independent dependency chains)."""
    n_chunks = N // nt
    nc = bacc.Bacc("TRN2", target_bir_lowering=False, debug=False,
                   num_devices=N_CORES)
    x = nc.dram_tensor("x", [B_PER_CORE, F, L, N], F32, kind="ExternalInput").ap()
    w2t = nc.dram_tensor("w2t", [128, 128], F32, kind="ExternalInput").ap()
    out = nc.dram_tensor("out", [B_PER_CORE, F, L, N], F32, kind="ExternalOutput").ap()

    xv = x.rearrange("b f l n -> (b f) l n")    # [128, 8, N]
    ov = out.rearrange("b f l n -> (b f) l n")

    with ExitStack() as ctx:
        tc = ctx.enter_context(tile.TileContext(nc))
        singles = ctx.enter_context(tc.tile_pool(name="singles", bufs=1))
        xpool = ctx.enter_context(tc.tile_pool(name="xp", bufs=3))
        dpool = ctx.enter_context(tc.tile_pool(name="dp", bufs=3))
        epool = ctx.enter_context(tc.tile_pool(name="ep", bufs=2))
        ppool = ctx.enter_context(tc.tile_pool(name="pp", bufs=2))
        kpool = ctx.enter_context(tc.tile_pool(name="kp", bufs=3))
        mpool = ctx.enter_context(tc.tile_pool(name="mp", bufs=3))
        opool = ctx.enter_context(tc.tile_pool(name="op", bufs=3))
        psum = ctx.enter_context(tc.tile_pool(name="ps", bufs=psum_bufs,
                                              space="PSUM"))

        w_sb = singles.tile([128, 128], F32)
        nc.sync.dma_start(out=w_sb[:], in_=w2t)

        for j in range(n_chunks):
            n0 = j * nt
            X = xpool.tile([128, L, nt], F32)
            nc.sync.dma_start(out=X[:], in_=xv[:, :, n0:n0 + nt])

            D_ps = psum.tile([128, L, nt], F32)
            per_mm = 512 // nt  # l-slices per 512-free matmul
            for k in range(L // per_mm):
                sl = slice(k * per_mm, (k + 1) * per_mm)
                nc.tensor.matmul(D_ps[:, sl, :], lhsT=w_sb[:], rhs=X[:, sl, :],
                                 start=True, stop=True)

            D = dpool.tile([128, L, nt], F32)
            nc.scalar.copy(D[:], D_ps[:])

            # E45 = [[2,1],[1,2]] @ D[4:6] along L
            E45 = epool.tile([128, 2, nt], F32)
            nc.vector.scalar_tensor_tensor(E45[:, 0, :], in0=D[:, 4, :],
                                           scalar=2.0, in1=D[:, 5, :],
                                           op0=MULT, op1=ADD)
            nc.vector.scalar_tensor_tensor(E45[:, 1, :], in0=D[:, 5, :],
                                           scalar=2.0, in1=D[:, 4, :],
                                           op0=MULT, op1=ADD)

            # P[l] = x[l] * (K d)[l]
            P = ppool.tile([128, L, nt], F32)
            nc.vector.tensor_mul(P[:, 0:2, :], X[:, 0:2, :], D[:, 6:8, :])
            nc.vector.tensor_mul(P[:, 2, :], X[:, 2, :], D[:, 3, :])
            nc.vector.tensor_mul(P[:, 3, :], X[:, 3, :], D[:, 2, :])
            nc.vector.tensor_mul(P[:, 4:6, :], X[:, 4:6, :], E45[:])
            nc.vector.tensor_mul(P[:, 6:8, :], X[:, 6:8, :], D[:, 0:2, :])

            kf = kpool.tile([128, nt], F32)
            nc.vector.tensor_reduce(kf[:], P[:].rearrange("p l n -> p n l"),
                                    axis=mybir.AxisListType.X, op=ADD)
            r = kpool.tile([128, nt], F32, tag="r")
            nc.scalar.activation(r[:], kf[:], mybir.ActivationFunctionType.Relu)

            # out = x + relu(kf)*d, split by l-range into two independent
            # chains: DVE handles l<lsplit, GPSIMD handles the rest.
            M = mpool.tile([128, L, nt], F32)
            O = opool.tile([128, L, nt], F32)
            s = lsplit
            if s > 0:
                nc.vector.tensor_mul(M[:, 0:s, :], D[:, 0:s, :],
                                     _bcast_l(r[:], s))
                nc.vector.tensor_add(O[:, 0:s, :], X[:, 0:s, :], M[:, 0:s, :])
            if s < L:
                rb = _bcast_l(r[:], L - s)
                nc.gpsimd.tensor_mul(M[:, s:L, :], D[:, s:L, :], rb)
                nc.gpsimd.tensor_add(O[:, s:L, :], X[:, s:L, :], M[:, s:L, :])

            nc.sync.dma_start(out=ov[:, :, n0:n0 + nt], in_=O[:])

    nc.finalize()
    return nc


_CACHED = {}
CFG = (256, 2, 2)  # (nt, lsplit, psum_bufs) — chosen by cost-model sweep


def _get_program(cfg=None):
    cfg = cfg or CFG
    if cfg not in _CACHED:
        _CACHED[cfg] = _build(*cfg)
    return _CACHED[cfg]


def _run(x: np.ndarray, W: np.ndarray, trace: bool = False, cfg=None):
    nc = _get_program(cfg)
    w2t = np.zeros((128, 128), dtype=np.float32)
    wt = np.ascontiguousarray(W.T.astype(np.float32))
    w2t[:64, :64] = wt
    w2t[64:, 64:] = wt
    in_maps = [
        {"x": np.ascontiguousarray(x[c * B_PER_CORE:(c + 1) * B_PER_CORE]),
         "w2t": w2t}
        for c in range(N_CORES)
    ]
    res = run_bass_kernel_spmd(nc, in_maps, list(range(N_CORES)), trace=trace)
    out = np.concatenate([res.results[c]["out"] for c in range(N_CORES)], axis=0)
    return out, res


def kernel(x: np.ndarray, W: np.ndarray) -> np.ndarray:
    out, _ = _run(np.asarray(x, dtype=np.float32), np.asarray(W, dtype=np.float32))
    return out



# revision 3
# speedup vs baseline: 1.4719x; 1.4719x over previous
"""Trainium2 Bass kernel for nn_LNKillingRelu (fp16 pipeline).

Math (per batch b, channel g, point n; L=8 lie-algebra coords):
    d[b,g,:,n]  = sum_f W[g,f] * x[b,f,:,n]          (64x64 linear over channels)
    kf[b,g,n]   = x[b,g,:,n]^T K d[b,g,:,n]          (8x8 Killing bilinear form)
    out         = x + relu(kf) * d                    (broadcast kf over L)

K is symmetric & sparse: pairs (0<->6, 1<->7, 2<->3) plus [[2,1],[1,2]] on
coords (4,5).  We use kf = sum_l (Kx)[l] * d[l] with
(Kx) = [x6, x7, x3, x2, 2x4+x5, x4+2x5, x0, x1].

Everything on-device runs in fp16 (tolerance is 2e-2 scale-relative; fp16
keeps it ~1e-3): halves DMA bytes (the binding resource - all DMA transfers
serialize at ~360B/ns aggregate) and doubles DVE throughput, and runs the
PE at 1 cycle/row instead of 4.  The host converts fp32<->fp16.

Sharding: data-parallel over batch B=16 -> 2 batches per core (8 cores).
Per-core layout: partitions = (batch-pair, F=64) = 128; free = (L, n-chunk).
PE computes d with a block-diag(W^T, W^T) 128x128 stationary fp16 weight.
ACT drains PSUM->SBUF (fp32->fp16); DVE+GPSIMD share the Killing products,
tree-reduce, and the out = x + relu(kf)*d tail.  Chunk sizes ramp up
(256..1024) to shorten pipeline head/tail while keeping per-op overhead low.
"""

import numpy as np
from contextlib import ExitStack

import concourse.bass as bass
import concourse.bacc as bacc
import concourse.tile as tile
from concourse import mybir
from concourse.bass_utils import run_bass_kernel_spmd

B, F, L, N = 16, 64, 8, 4096
N_CORES = 8
B_PER_CORE = B // N_CORES  # 2

F16 = mybir.dt.float16
F32 = mybir.dt.float32
MULT = mybir.AluOpType.mult
ADD = mybir.AluOpType.add

NQ = 256  # psum quarter width (4 banks fp32 at L=8)


def _bcast_l(ap: bass.AP, l: int) -> bass.AP:
    """[128, NT] AP -> [128, l, NT] with a zero-stride middle dim."""
    return bass.AP(tensor=ap.tensor, offset=ap.offset,
                   ap=[ap.ap[0], [0, l], ap.ap[1]])


# default engine assignment: 'v' = DVE, 'p' = GPSIMD(Pool)
DEFAULT_ASSIGN = dict(pa='v', pb='v', pc='p', pd='v', pe='v',
                      xe0='p', xe1='p', l1='v', l2='v', l3='v',
                      osplit=3, pipe=1)
DEFAULT_CHUNKS = (512,) * 8


def _build(chunks=DEFAULT_CHUNKS, assign=None):
    asn = dict(DEFAULT_ASSIGN)
    if assign:
        asn.update(assign)
    assert sum(chunks) == N
    nc = bacc.Bacc("TRN2", target_bir_lowering=False, debug=False,
                   num_devices=N_CORES)
    x = nc.dram_tensor("x", [B_PER_CORE, F, L, N], F16, kind="ExternalInput").ap()
    w2t = nc.dram_tensor("w2t", [128, 128], F16, kind="ExternalInput").ap()
    out = nc.dram_tensor("out", [B_PER_CORE, F, L, N], F16, kind="ExternalOutput").ap()

    xv = x.rearrange("b f l n -> (b f) l n")    # [128, 8, N]
    ov = out.rearrange("b f l n -> (b f) l n")

    def eng(key):
        return nc.vector if asn[key] == 'v' else nc.gpsimd

    with ExitStack() as ctx:
        tc = ctx.enter_context(tile.TileContext(nc))
        singles = ctx.enter_context(tc.tile_pool(name="singles", bufs=1))
        xpool = ctx.enter_context(tc.tile_pool(name="xp", bufs=2))
        dpool = ctx.enter_context(tc.tile_pool(name="dp", bufs=2))
        vpool = ctx.enter_context(tc.tile_pool(name="vp", bufs=2))
        tpool = ctx.enter_context(tc.tile_pool(name="tp", bufs=2))
        mpool = ctx.enter_context(tc.tile_pool(name="mp", bufs=2))
        opool = ctx.enter_context(tc.tile_pool(name="op", bufs=2))
        rpool = ctx.enter_context(tc.tile_pool(name="rp", bufs=2))
        psum = ctx.enter_context(tc.tile_pool(name="ps", bufs=2, space="PSUM"))

        w_sb = singles.tile([128, 128], F16)
        nc.sync.dma_start(out=w_sb[:], in_=w2t)

        pending = []  # deferred out-stage: (X, D, r, n0, nt)

        def emit_out_stage(X, D, r, n0, nt):
            M = mpool.tile([128, L, nt], F16, tag=f"M{nt}")
            O = opool.tile([128, L, nt], F16, tag=f"O{nt}")
            s = asn['osplit']
            if s > 0:
                nc.vector.tensor_mul(M[:, 0:s, :], D[:, 0:s, :], _bcast_l(r[:], s))
                nc.vector.tensor_add(O[:, 0:s, :], X[:, 0:s, :], M[:, 0:s, :])
            if s < L:
                rb = _bcast_l(r[:], L - s)
                nc.gpsimd.tensor_mul(M[:, s:L, :], D[:, s:L, :], rb)
                nc.gpsimd.tensor_add(O[:, s:L, :], X[:, s:L, :], M[:, s:L, :])
            nc.sync.dma_start(out=ov[:, :, n0:n0 + nt], in_=O[:])

        n0 = 0
        for nt in chunks:
            X = xpool.tile([128, L, nt], F16, tag=f"X{nt}")
            nc.sync.dma_start(out=X[:], in_=xv[:, :, n0:n0 + nt])

            # Xe = (2x4+x5, x4+2x5): only needs X, fills Pool early.
            Xe = tpool.tile([128, 2, nt], F16, tag=f"Xe{nt}")
            eng('xe0').scalar_tensor_tensor(Xe[:, 0, :], in0=X[:, 4, :],
                                            scalar=2.0, in1=X[:, 5, :],
                                            op0=MULT, op1=ADD)
            eng('xe1').scalar_tensor_tensor(Xe[:, 1, :], in0=X[:, 5, :],
                                            scalar=2.0, in1=X[:, 4, :],
                                            op0=MULT, op1=ADD)

            # D = W @ X via psum quarter-tiles (4 banks each, double buffered)
            D = dpool.tile([128, L, nt], F16, tag=f"D{nt}")
            for q in range(nt // NQ):
                c0 = q * NQ
                ps = psum.tile([128, L, NQ], F32, tag="ps")
                for k in range(L // 2):
                    sl = slice(2 * k, 2 * k + 2)
                    nc.tensor.matmul(ps[:, sl, :], lhsT=w_sb[:],
                                     rhs=X[:, sl, c0:c0 + NQ],
                                     start=True, stop=True)
                nc.scalar.copy(D[:, :, c0:c0 + NQ], ps[:])

            # V[l] = (Kx)[l] * D[l]
            V = vpool.tile([128, L, nt], F16, tag=f"V{nt}")
            eng('pa').tensor_mul(V[:, 0:2, :], X[:, 6:8, :], D[:, 0:2, :])
            eng('pb').tensor_mul(V[:, 2, :], X[:, 3, :], D[:, 2, :])
            eng('pc').tensor_mul(V[:, 3, :], X[:, 2, :], D[:, 3, :])
            eng('pd').tensor_mul(V[:, 4:6, :], Xe[:], D[:, 4:6, :])
            eng('pe').tensor_mul(V[:, 6:8, :], X[:, 0:2, :], D[:, 6:8, :])

            # kf = sum_l V[l] via tree adds (tensor_tensor keeps the fp16 2x
            # DVE mode; tensor_reduce over strided l would run at full rate)
            T4 = tpool.tile([128, 4, nt], F16, tag=f"T4{nt}")
            T2 = tpool.tile([128, 2, nt], F16, tag=f"T2{nt}")
            kf = rpool.tile([128, nt], F16, tag=f"kf{nt}")
            eng('l1').tensor_add(T4[:], V[:, 0:4, :], V[:, 4:8, :])
            eng('l2').tensor_add(T2[:], T4[:, 0:2, :], T4[:, 2:4, :])
            eng('l3').tensor_add(kf[:], T2[:, 0, :], T2[:, 1, :])
            r = rpool.tile([128, nt], F16, tag=f"r{nt}")
            nc.scalar.activation(r[:], kf[:], mybir.ActivationFunctionType.Relu)

            if asn['pipe']:
                pending.append((X, D, r, n0, nt))
                if len(pending) > 1:
                    emit_out_stage(*pending.pop(0))
            else:
                emit_out_stage(X, D, r, n0, nt)
            n0 += nt

        for args in pending:
            emit_out_stage(*args)

    nc.finalize()
    return nc


_CACHED = {}
CFG = (DEFAULT_CHUNKS, ())


def _freeze(cfg):
    chunks, assign = cfg
    return (tuple(chunks), tuple(sorted(dict(assign).items())))


def _get_program(cfg=None):
    cfg = cfg or CFG
    key = _freeze(cfg)
    if key not in _CACHED:
        _CACHED[key] = _build(tuple(cfg[0]), dict(cfg[1]))
    return _CACHED[key]


def _run(x: np.ndarray, W: np.ndarray, trace: bool = False, cfg=None):
    nc = _get_program(cfg)
    w2t = np.zeros((128, 128), dtype=np.float16)
    wt = np.ascontiguousarray(W.T).astype(np.float16)
    w2t[:64, :64] = wt
    w2t[64:, 64:] = wt
    x16 = np.asarray(x, dtype=np.float16)
    in_maps = [
        {"x": np.ascontiguousarray(x16[c * B_PER_CORE:(c + 1) * B_PER_CORE]),
         "w2t": w2t}
        for c in range(N_CORES)
    ]
    res = run_bass_kernel_spmd(nc, in_maps, list(range(N_CORES)), trace=trace)
    out = np.concatenate([res.results[c]["out"] for c in range(N_CORES)], axis=0)
    return out.astype(np.float32), res


def kernel(x: np.ndarray, W: np.ndarray) -> np.ndarray:
    out, _ = _run(np.asarray(x, dtype=np.float32), np.asarray(W, dtype=np.float32))
    return out


# revision 6
# speedup vs baseline: 2.0173x; 1.3705x over previous
"""Trainium2 Bass kernel for nn_LNKillingRelu (fp16 pipeline).

Math (per batch b, channel g, point n; L=8 lie-algebra coords):
    d[b,g,:,n]  = sum_f W[g,f] * x[b,f,:,n]          (64x64 linear over channels)
    kf[b,g,n]   = x[b,g,:,n]^T K d[b,g,:,n]          (8x8 Killing bilinear form)
    out         = x + relu(kf) * d                    (broadcast kf over L)

K is symmetric & sparse: pairs (0<->6, 1<->7, 2<->3) plus [[2,1],[1,2]] on
coords (4,5).  We use kf = sum_l (Kx)[l] * d[l] with
(Kx) = [x6, x7, x3, x2, 2x4+x5, x4+2x5, x0, x1].

Everything on-device runs in fp16 (tolerance is 2e-2 scale-relative; fp16
keeps it ~1e-3): halves DMA bytes (the binding resource - all DMA transfers
serialize at ~360B/ns aggregate) and doubles DVE throughput, and runs the
PE at 1 cycle/row instead of 4.  The host converts fp32<->fp16.

Sharding: data-parallel over batch B=16 -> 2 batches per core (8 cores).
Per-core layout: partitions = (batch-pair, F=64) = 128; free = (L, n-chunk).
PE computes d with a block-diag(W^T, W^T) 128x128 stationary fp16 weight.
ACT drains PSUM->SBUF (fp32->fp16); DVE+GPSIMD share the Killing products,
tree-reduce, and the out = x + relu(kf)*d tail.  Chunk sizes ramp up
(256..1024) to shorten pipeline head/tail while keeping per-op overhead low.
"""

import numpy as np
from contextlib import ExitStack

import concourse.bass as bass
import concourse.bacc as bacc
import concourse.tile as tile
from concourse import mybir
from concourse.bass_utils import run_bass_kernel_spmd

B, F, L, N = 16, 64, 8, 4096
N_CORES = 8
B_PER_CORE = B // N_CORES  # 2

F16 = mybir.dt.float16
F32 = mybir.dt.float32
MULT = mybir.AluOpType.mult
ADD = mybir.AluOpType.add

NQ = 256  # psum quarter width (4 banks fp32 at L=8)


def _bcast_l(ap: bass.AP, l: int) -> bass.AP:
    """[128, NT] AP -> [128, l, NT] with a zero-stride middle dim."""
    return bass.AP(tensor=ap.tensor, offset=ap.offset,
                   ap=[ap.ap[0], [0, l], ap.ap[1]])


# default engine assignment: 'v' = DVE, 'p' = GPSIMD(Pool)
DEFAULT_ASSIGN = dict(pa='v', pb='v', pc='p', pd='v', pe='v',
                      xe0='p', xe1='p', l1='v', l2='v', l3='v',
                      osplit=3, pipe=1)
DEFAULT_CHUNKS = (512,) * 8


def _build(chunks=DEFAULT_CHUNKS, assign=None):
    asn = dict(DEFAULT_ASSIGN)
    if assign:
        asn.update(assign)
    assert sum(chunks) == N
    nc = bacc.Bacc("TRN2", target_bir_lowering=False, debug=False,
                   num_devices=N_CORES)
    x = nc.dram_tensor("x", [B_PER_CORE, F, L, N], F16, kind="ExternalInput").ap()
    w2t = nc.dram_tensor("w2t", [128, 128], F16, kind="ExternalInput").ap()
    out = nc.dram_tensor("out", [B_PER_CORE, F, L, N], F16, kind="ExternalOutput").ap()

    xv = x.rearrange("b f l n -> (b f) l n")    # [128, 8, N]
    ov = out.rearrange("b f l n -> (b f) l n")

    def eng(key):
        return nc.vector if asn[key] == 'v' else nc.gpsimd

    with ExitStack() as ctx:
        tc = ctx.enter_context(tile.TileContext(nc))
        singles = ctx.enter_context(tc.tile_pool(name="singles", bufs=1))
        xpool = ctx.enter_context(tc.tile_pool(name="xp", bufs=3))
        dpool = ctx.enter_context(tc.tile_pool(name="dp", bufs=3))
        vpool = ctx.enter_context(tc.tile_pool(name="vp", bufs=2))
        tpool = ctx.enter_context(tc.tile_pool(name="tp", bufs=2))
        mpool = ctx.enter_context(tc.tile_pool(name="mp", bufs=2))
        opool = ctx.enter_context(tc.tile_pool(name="op", bufs=2))
        rpool = ctx.enter_context(tc.tile_pool(name="rp", bufs=3))
        psum = ctx.enter_context(tc.tile_pool(name="ps", bufs=2, space="PSUM"))

        w_sb = singles.tile([128, 128], F16)
        nc.sync.dma_start(out=w_sb[:], in_=w2t)

        pending = []  # deferred out-stage: (X, D, r, n0, nt)

        def emit_out_stage(X, D, r, n0, nt):
            M = mpool.tile([128, L, nt], F16, tag=f"M{nt}")
            O = opool.tile([128, L, nt], F16, tag=f"O{nt}")
            s = asn['osplit']
            if s > 0:
                nc.vector.tensor_mul(M[:, 0:s, :], D[:, 0:s, :], _bcast_l(r[:], s))
                nc.vector.tensor_add(O[:, 0:s, :], X[:, 0:s, :], M[:, 0:s, :])
            if s < L:
                rb = _bcast_l(r[:], L - s)
                nc.gpsimd.tensor_mul(M[:, s:L, :], D[:, s:L, :], rb)
                nc.gpsimd.tensor_add(O[:, s:L, :], X[:, s:L, :], M[:, s:L, :])
            nc.sync.dma_start(out=ov[:, :, n0:n0 + nt], in_=O[:])

        n0 = 0
        for nt in chunks:
            # in-DMA split by column halves: the first half's matmuls can
            # start while the second half is still transferring.
            X = xpool.tile([128, L, nt], F16, tag=f"X{nt}")
            nh = nt // 2
            nc.sync.dma_start(out=X[:, :, 0:nh], in_=xv[:, :, n0:n0 + nh])
            nc.sync.dma_start(out=X[:, :, nh:nt], in_=xv[:, :, n0 + nh:n0 + nt])

            # Xe = (2x4+x5, x4+2x5): only needs X, fills Pool early.
            Xe = tpool.tile([128, 2, nt], F16, tag=f"Xe{nt}")
            eng('xe0').scalar_tensor_tensor(Xe[:, 0, :], in0=X[:, 4, :],
                                            scalar=2.0, in1=X[:, 5, :],
                                            op0=MULT, op1=ADD)
            eng('xe1').scalar_tensor_tensor(Xe[:, 1, :], in0=X[:, 5, :],
                                            scalar=2.0, in1=X[:, 4, :],
                                            op0=MULT, op1=ADD)

            # D = W @ X via psum quarter-tiles (4 banks each, double buffered)
            D = dpool.tile([128, L, nt], F16, tag=f"D{nt}")
            for q in range(nt // NQ):
                c0 = q * NQ
                ps = psum.tile([128, L, NQ], F32, tag="ps")
                for k in range(L // 2):
                    sl = slice(2 * k, 2 * k + 2)
                    nc.tensor.matmul(ps[:, sl, :], lhsT=w_sb[:],
                                     rhs=X[:, sl, c0:c0 + NQ],
                                     start=True, stop=True)
                nc.scalar.copy(D[:, :, c0:c0 + NQ], ps[:])

            # deferred out-stage of the previous chunk goes here: it is
            # ready to run (r already computed) and fills the gap while
            # this chunk's D is still in the PE/ACT pipe.
            if asn['pipe'] and pending:
                emit_out_stage(*pending.pop(0))

            # V[l] = (Kx)[l] * D[l]
            V = vpool.tile([128, L, nt], F16, tag=f"V{nt}")
            eng('pa').tensor_mul(V[:, 0:2, :], X[:, 6:8, :], D[:, 0:2, :])
            eng('pb').tensor_mul(V[:, 2, :], X[:, 3, :], D[:, 2, :])
            eng('pc').tensor_mul(V[:, 3, :], X[:, 2, :], D[:, 3, :])
            eng('pd').tensor_mul(V[:, 4:6, :], Xe[:], D[:, 4:6, :])
            eng('pe').tensor_mul(V[:, 6:8, :], X[:, 0:2, :], D[:, 6:8, :])

            # kf = sum_l V[l] via tree adds (tensor_tensor keeps the fp16 2x
            # DVE mode; tensor_reduce over strided l would run at full rate)
            T4 = tpool.tile([128, 4, nt], F16, tag=f"T4{nt}")
            T2 = tpool.tile([128, 2, nt], F16, tag=f"T2{nt}")
            kf = rpool.tile([128, nt], F16, tag=f"kf{nt}")
            eng('l1').tensor_add(T4[:], V[:, 0:4, :], V[:, 4:8, :])
            eng('l2').tensor_add(T2[:], T4[:, 0:2, :], T4[:, 2:4, :])
            eng('l3').tensor_add(kf[:], T2[:, 0, :], T2[:, 1, :])
            r = rpool.tile([128, nt], F16, tag=f"r{nt}")
            nc.scalar.activation(r[:], kf[:], mybir.ActivationFunctionType.Relu)

            if asn['pipe']:
                pending.append((X, D, r, n0, nt))
            else:
                emit_out_stage(X, D, r, n0, nt)
            n0 += nt

        for args in pending:
            emit_out_stage(*args)

    nc.finalize()
    return nc


_CACHED = {}
CFG = (DEFAULT_CHUNKS, ())


def _freeze(cfg):
    chunks, assign = cfg
    return (tuple(chunks), tuple(sorted(dict(assign).items())))


def _get_program(cfg=None):
    cfg = cfg or CFG
    key = _freeze(cfg)
    if key not in _CACHED:
        _CACHED[key] = _build(tuple(cfg[0]), dict(cfg[1]))
    return _CACHED[key]


def _run(x: np.ndarray, W: np.ndarray, trace: bool = False, cfg=None):
    nc = _get_program(cfg)
    w2t = np.zeros((128, 128), dtype=np.float16)
    wt = np.ascontiguousarray(W.T).astype(np.float16)
    w2t[:64, :64] = wt
    w2t[64:, 64:] = wt
    x16 = np.asarray(x, dtype=np.float16)
    in_maps = [
        {"x": np.ascontiguousarray(x16[c * B_PER_CORE:(c + 1) * B_PER_CORE]),
         "w2t": w2t}
        for c in range(N_CORES)
    ]
    res = run_bass_kernel_spmd(nc, in_maps, list(range(N_CORES)), trace=trace)
    out = np.concatenate([res.results[c]["out"] for c in range(N_CORES)], axis=0)
    return out.astype(np.float32), res


def kernel(x: np.ndarray, W: np.ndarray) -> np.ndarray:
    out, _ = _run(np.asarray(x, dtype=np.float32), np.asarray(W, dtype=np.float32))
    return out
